# revision 33
# baseline (speedup 1.0000x reference)
"""Variable-length average pooling (prefix mean over seq axis) on 8 trn2 cores.

Strategy (pure data parallelism over batch, dense packed layout):
  - eff_len[b] = lengths[b] if >0 else L.  pooled[b] = sum_{l<eff} x[b,l,:] / eff.
  - Sort batches by eff_len desc, snake-assign 16 per core; slot s's baked
    row budget is smax[s] = max over cores of that slot's eff (sorted
    grouping keeps the spread small, ~5% over the per-core ideal).
  - Host packs each core's rows 0..smax[s] of every slot back-to-back into
    one dense [T, D] buffer (T = sum smax ~ 9019 rows, 73.9 MB vs the 82 MB
    a 128-aligned per-slot layout loads).  Slots are ordered by ascending
    smax: the tiny-slot psum churn lands in the DMA ramp-up phase and the
    kernel drains on one long all-PE slot (one matmul group + copy + out).
  - The SPMD program streams T rows as 128-row chunks (2 MB pair DMAs
    alternating the two HWDGE rings SP/ACT; 16 SDMA engines saturate at
    ~360-370 GB/s unthrottled, so the DMA floor is ~203 us and every
    compute engine must stay under it).  The first two pairs are issued
    before the mask DMAs; 10 pair buffers (20 MB SBUF) ride out transient
    consumer backlogs that otherwise stall the rings on buffer reuse.
  - Tensors are declared float32r: at moving free dim 512 the PE runs
    1 cycle/row vs 4 for exact fp32 (bits are identical to fp32 on the
    host; extra rounding error measured at 1e-4 vs the 2e-2 gate).  A DMA
    into an f32r dram tensor satisfies the BIR verifier's "rounded to
    FP32r" rule; bitcasting an f32 tile at the matmul does not.
  - Work split so no engine approaches the DMA floor (measured: PE 2.36us
    per 4-matmul fragment group, DVE add 2.28us, all-PE was 206us busy):
      * chunks fully inside one slot and below the slot's min core eff
        ("uniform": every row valid on every core) accumulate into a
        per-slot SBUF tile on the DVE; one PE pass per slot (ones-column
        matmul, issued at slot end -- issuing it earlier serializes the
        PE queue behind the DVE add chain, measured +12us) reduces it.
      * boundary / ragged fragments go straight to the PE as
        psum[1,512] += col[128,1].T @ chunk[128,512] with host-built 0/1
        columns (zero outside the fragment or past the core's own eff),
        accumulated across a slot's chunks in two 2-bank psum tiles
        (a single 4-bank [1,2048] psum tile breaks accumulation).
  - PSUM stays fp32; the 1/eff scale is applied per-core-exactly at the
    PSUM->SBUF copy on the DVE (Pool cannot access PSUM; anything queued
    on ACT blocks its ring-B load issues, measured +19us of DMA gaps).
  - Measured 217-220us unthrottled (vs 256-269us baseline); the device
    power-throttles DMA to ~310-330 GB/s when hot, adding up to ~25us
    run-to-run -- compute sits at <=145us per engine so the kernel stays
    DMA-bound either way.
"""

import os

import numpy as np

import concourse.bacc as bacc
import concourse.mybir as mybir
from concourse.tile import TileContext
from concourse.bass_utils import run_bass_kernel_spmd

B, L, D = 128, 1024, 2048
NCORES = 8
SLOTS = B // NCORES  # 16
PCHUNK = 128         # rows per chunk (partition dim of a tile)
NTILE = 512          # matmul moving free dim (one PSUM bank of fp32)

TILE_BUFS = int(os.environ.get("TILE_BUFS", "10"))
KCAP = int(os.environ.get("KCAP", "48"))  # max chunks on the DVE add path
REDUCE_EARLY = os.environ.get("REDUCE_EARLY", "0") == "1"

LAST_RESULTS = None  # BassKernelResults of the most recent device run


def _plan(eff):
    """Snake-assign sorted batches to cores; order slots by ascending max."""
    order = np.argsort(-eff, kind="stable")
    cores = [[] for _ in range(NCORES)]
    for i, idx in enumerate(order):
        blk, pos = divmod(i, NCORES)
        c = pos if blk % 2 == 0 else NCORES - 1 - pos
        cores[c].append(int(idx))
    smax = [max(int(eff[cores[c][s]]) for c in range(NCORES)) for s in range(SLOTS)]
    smin = [min(int(eff[cores[c][s]]) for c in range(NCORES)) for s in range(SLOTS)]
    sorder = sorted(range(SLOTS), key=lambda s: smax[s])
    cores = [[cores[c][s] for s in sorder] for c in range(NCORES)]
    smax = tuple(smax[s] for s in sorder)
    smin = tuple(smin[s] for s in sorder)
    return cores, smax, smin


def _fragments(smax, smin):
    """Chunk the packed [T, D] stream into (chunk, slot) fragments.

    Returns (T, offsets g, frags, n_pe).  frags[i] = (c, s, a, b, r0, fi):
    partitions [a,b) of chunk c hold slot-local rows [r0, r0 + b - a) of
    slot s; fi is the mask-column index for PE fragments or None for
    uniform chunks that take the DVE-accumulator path."""
    g = [0]
    for m in smax:
        g.append(g[-1] + m)
    T = g[-1]
    nchunk = -(-T // PCHUNK)
    raw, s = [], 0
    for c in range(nchunk):
        lo, hi = c * PCHUNK, min((c + 1) * PCHUNK, T)
        while g[s + 1] <= lo:
            s += 1
        si = s
        while si < SLOTS and g[si] < hi:
            a = max(lo, g[si]) - lo
            b = min(hi, g[si + 1]) - lo
            raw.append((c, si, a, b, max(lo, g[si]) - g[si]))
            si += 1
    # Uniform chunks: whole 128-row chunk inside one slot, and every core
    # has all its rows valid (r0 + 128 <= smin) -- no mask needed.  The
    # final slot stays entirely on the PE so the drain after the last DMA
    # is one matmul group + scale + out, not an add+reduce chain.
    uni = [
        i
        for i, (c, s, a, b, r0) in enumerate(raw)
        if b - a == PCHUNK and r0 + PCHUNK <= smin[s] and s < SLOTS - 1
    ]
    if len(uni) > KCAP:  # keep the DVE under the DMA floor; demote evenly
        keep = {uni[(i * len(uni)) // KCAP] for i in range(KCAP)}
        uni = [i for i in uni if i in keep]
    uni = set(uni)
    frags, n_pe = [], 0
    for i, (c, s, a, b, r0) in enumerate(raw):
        if i in uni:
            frags.append((c, s, a, b, r0, None))
        else:
            frags.append((c, s, a, b, r0, n_pe))
            n_pe += 1
    return T, g, frags, n_pe


_PROGRAM_CACHE = {}


def _build_program(smax, smin):
    T, _, frags, n_pe = _fragments(smax, smin)
    ones_col = n_pe  # shared all-ones column for accumulator reduces
    by_chunk = {}
    for c, s, a, b, r0, fi in frags:
        by_chunk.setdefault(c, []).append((s, a, b, r0, fi))
    slot_last_chunk = {}
    slot_last_uni = {}  # slot -> chunk of its last uniform (DVE-path) chunk
    for c, s, a, b, r0, fi in frags:
        slot_last_chunk[s] = max(slot_last_chunk.get(s, -1), c)
        if fi is None:
            slot_last_uni[s] = max(slot_last_uni.get(s, -1), c)
    if not REDUCE_EARLY:  # reduce at slot end instead
        slot_last_uni = {s: slot_last_chunk[s] for s in slot_last_uni}
    # The slot's final PE matmul (gets stop=True): the reduce if no PE
    # fragment follows the last uniform chunk, else the last fragment.
    slot_stop_on_reduce = {
        s: slot_last_uni[s] == slot_last_chunk[s] for s in slot_last_uni
    }

    # Bacc (not raw Bass): its compile pass splits multi-sem waits and moves
    # matmul waits onto ldweights -- walrus allows only 1 wait per instruction.
    nc = bacc.Bacc(None, target_bir_lowering=False)
    f32 = mybir.dt.float32
    f32r = mybir.dt.float32r
    packed = nc.dram_tensor("packed", [T, D], f32r, kind="ExternalInput")
    maskt = nc.dram_tensor("maskt", [PCHUNK, n_pe + 1], f32r, kind="ExternalInput")
    scalef = nc.dram_tensor("scalef", [1, SLOTS], f32, kind="ExternalInput")
    out = nc.dram_tensor("out", [SLOTS, D], f32, kind="ExternalOutput")

    with TileContext(nc) as tc:
        with (
            tc.tile_pool(name="mask", bufs=1) as mpool,
            tc.tile_pool(name="scale", bufs=1) as spool,
            tc.tile_pool(name="tiles", bufs=TILE_BUFS) as tpool,
            tc.tile_pool(name="accs", bufs=2) as apool,
            tc.tile_pool(name="psum", bufs=4, space="PSUM") as ppool,
            tc.tile_pool(name="outs", bufs=3) as opool,
        ):
            mask_tile = mpool.tile([PCHUNK, n_pe + 1], f32r)
            scale_tile = spool.tile([1, SLOTS], f32)
            dma_engines = [nc.sync, nc.scalar]
            psums = {}    # slot -> (psum_a, psum_b)
            started = set()
            accs = {}     # slot -> acc tile

            def mm_group(s, lhsT, rhs_tile, off, rows, stop):
                if s not in psums:
                    psums[s] = (
                        ppool.tile([1, D // 2], f32, name="psum_a", tag="ps"),
                        ppool.tile([1, D // 2], f32, name="psum_b", tag="ps"),
                    )
                pa, pb = psums[s]
                half = [pa, pa, pb, pb]
                start = s not in started
                started.add(s)
                for j in range(D // NTILE):
                    nc.tensor.matmul(
                        half[j][0:1, (j % 2) * NTILE : (j % 2 + 1) * NTILE],
                        lhsT,
                        rhs_tile[0:rows, off + j * NTILE : off + (j + 1) * NTILE],
                        start=start,
                        stop=stop,
                    )

            def consume_chunk(c, tile, off, rows):
                done = []
                for s, a, b, r0, fi in by_chunk[c]:
                    if fi is None:
                        if s not in accs:
                            accs[s] = apool.tile([PCHUNK, D], f32r, name="acc", tag="a")
                            nc.vector.tensor_copy(
                                out=accs[s][:], in_=tile[:, off : off + D]
                            )
                        else:
                            nc.vector.tensor_add(
                                out=accs[s][:],
                                in0=accs[s][:],
                                in1=tile[:, off : off + D],
                            )
                    else:
                        mm_group(
                            s,
                            mask_tile[0:rows, fi : fi + 1],
                            tile,
                            off,
                            rows,
                            stop=(
                                c == slot_last_chunk[s]
                                and not slot_stop_on_reduce.get(s, False)
                            ),
                        )
                    if c == slot_last_uni.get(s, -1):
                        # Reduce as soon as the accumulator is complete --
                        # trailing PE fragments then overlap it, shortening
                        # the post-final-DMA drain chain.
                        mm_group(
                            s,
                            mask_tile[0:PCHUNK, ones_col : ones_col + 1],
                            accs.pop(s),
                            0,
                            PCHUNK,
                            stop=slot_stop_on_reduce[s],
                        )
                    if c == slot_last_chunk[s]:
                        done.append(s)
                for s in done:
                    pa, pb = psums.pop(s)
                    # DVE for PSUM->SBUF (Pool can't access PSUM, and any
                    # op queued on ACT blocks its ring-B load issues -- that
                    # cost 19us of mid-run DMA gaps when measured).
                    sc = scale_tile[0:1, s : s + 1]
                    ot = opool.tile([1, D], f32)
                    nc.vector.tensor_scalar_mul(
                        out=ot[:, 0 : D // 2], in0=pa[:], scalar1=sc
                    )
                    nc.vector.tensor_scalar_mul(
                        out=ot[:, D // 2 : D], in0=pb[:], scalar1=sc
                    )
                    nc.sync.dma_start(out=out[s : s + 1, :], in_=ot[:])

            def load_pair(c):
                pair = tpool.tile([PCHUNK, 2 * D], f32r, name="pair", tag="t")
                src = packed[c * PCHUNK : (c + 2) * PCHUNK, :].rearrange(
                    "(c p) d -> p c d", p=PCHUNK
                )
                dst = pair[:].rearrange("p (c d) -> p c d", c=2)
                dma_engines[c // 2 % 2].dma_start(out=dst, in_=src)
                return pair

            c = 0
            nchunk = -(-T // PCHUNK)
            if T >= 4 * PCHUNK:
                # Issue the first two pair loads before the mask DMAs: the
                # big loads start ~3us earlier, and the first matmuls (tiny
                # early slots) have slack to wait for the masks behind them.
                p0, p1 = load_pair(0), load_pair(2)
                nc.sync.dma_start(out=mask_tile[:], in_=maskt[:])
                nc.scalar.dma_start(out=scale_tile[:], in_=scalef[:])
                for cc, (pp, off) in enumerate(
                    [(p0, 0), (p0, D), (p1, 0), (p1, D)]
                ):
                    consume_chunk(cc, pp, off, PCHUNK)
                c = 4
            else:
                nc.sync.dma_start(out=mask_tile[:], in_=maskt[:])
                nc.scalar.dma_start(out=scale_tile[:], in_=scalef[:])
            while c < nchunk:
                if T - c * PCHUNK >= 2 * PCHUNK:
                    pair = load_pair(c)
                    consume_chunk(c, pair, 0, PCHUNK)
                    consume_chunk(c + 1, pair, D, PCHUNK)
                    c += 2
                else:
                    rows = T - c * PCHUNK
                    single = tpool.tile([PCHUNK, D], f32r, name="single", tag="t")
                    dma_engines[c // 2 % 2].dma_start(
                        out=single[0:rows, :],
                        in_=packed[c * PCHUNK : c * PCHUNK + rows, :],
                    )
                    consume_chunk(c, single, 0, rows)
                    c += 1
    nc.finalize()
    return nc


def kernel(features, lengths):
    global LAST_RESULTS
    features = np.ascontiguousarray(features, dtype=np.float32)
    lengths = np.ascontiguousarray(lengths, dtype=np.int32)
    eff = np.where(lengths > 0, lengths, L).astype(np.int64)

    cores, smax, smin = _plan(eff)
    key = (smax, smin, TILE_BUFS, KCAP)
    if key not in _PROGRAM_CACHE:
        _PROGRAM_CACHE[key] = _build_program(smax, smin)
    nc = _PROGRAM_CACHE[key]
    T, g, frags, n_pe = _fragments(smax, smin)

    in_maps = []
    for c in range(NCORES):
        perm = cores[c]
        packed = np.empty((T, D), dtype=np.float32)
        for s, bi in enumerate(perm):
            packed[g[s] : g[s + 1]] = features[bi, : smax[s], :]
        maskt = np.zeros((PCHUNK, n_pe + 1), dtype=np.float32)
        for _, s, a, b, r0, fi in frags:
            if fi is None:
                continue
            v = min(max(int(eff[perm[s]]) - r0, 0), b - a)
            if v > 0:
                maskt[a : a + v, fi] = 1.0
        maskt[:, n_pe] = 1.0
        scalef = np.zeros((1, SLOTS), dtype=np.float32)
        for s in range(SLOTS):
            scalef[0, s] = np.float32(1.0 / eff[perm[s]])
        in_maps.append({"packed": packed, "maskt": maskt, "scalef": scalef})

    trace = os.environ.get("KERNEL_TRACE", "0") == "1"
    LAST_RESULTS = run_bass_kernel_spmd(
        nc,
        in_maps,
        core_ids=list(range(NCORES)),
        trace=trace,
        trace_cores=[0] if trace else None,
    )

    out = np.empty((B, D), dtype=np.float32)
    for c in range(NCORES):
        out[np.asarray(cores[c])] = LAST_RESULTS.results[c]["out"]
    return out


# revision 45
# speedup vs baseline: 1.0786x; 1.0786x over previous
"""Variable-length average pooling (prefix mean over seq axis) on 8 trn2 cores.

Strategy (pure data parallelism over batch, dense packed layout):
  - eff_len[b] = lengths[b] if >0 else L.  pooled[b] = sum_{l<eff} x[b,l,:] / eff.
  - Sort batches by eff_len desc, snake-assign 16 per core; slot s's baked
    row budget is smax[s] = max over cores of that slot's eff (sorted
    grouping keeps the spread small, ~5% over the per-core ideal).
  - Host packs each core's rows 0..smax[s] of every slot back-to-back into
    one dense [T, D] buffer (T = sum smax ~ 9019 rows, 73.9 MB vs the 82 MB
    a 128-aligned per-slot layout loads).  Slots are ordered by ascending
    smax: the tiny-slot psum churn lands in the DMA ramp-up phase and the
    kernel drains on one long all-PE slot (one matmul group + copy + out).
  - The SPMD program streams T rows as 128-row chunks (2 MB pair DMAs
    alternating the two HWDGE rings SP/ACT; 16 SDMA engines saturate at
    ~360-370 GB/s unthrottled, so the DMA floor is ~203 us and every
    compute engine must stay under it).  The first two pairs are issued
    before the mask DMAs; 10 pair buffers (20 MB SBUF) ride out transient
    consumer backlogs that otherwise stall the rings on buffer reuse.
  - Tensors are declared float32r: at moving free dim 512 the PE runs
    1 cycle/row vs 4 for exact fp32 (bits are identical to fp32 on the
    host; extra rounding error measured at 1e-4 vs the 2e-2 gate).  A DMA
    into an f32r dram tensor satisfies the BIR verifier's "rounded to
    FP32r" rule; bitcasting an f32 tile at the matmul does not.
  - Work split so no engine approaches the DMA floor (measured: PE 2.36us
    per 4-matmul fragment group, DVE add 2.28us, all-PE was 206us busy):
      * chunks fully inside one slot and below the slot's min core eff
        ("uniform": every row valid on every core) accumulate into a
        per-slot SBUF tile on the DVE; one PE pass per slot (ones-column
        matmul, issued at slot end -- issuing it earlier serializes the
        PE queue behind the DVE add chain, measured +12us) reduces it.
      * boundary / ragged fragments go straight to the PE as
        psum[1,512] += col[128,1].T @ chunk[128,512] with host-built 0/1
        columns (zero outside the fragment or past the core's own eff),
        accumulated across a slot's chunks in two 2-bank psum tiles
        (a single 4-bank [1,2048] psum tile breaks accumulation).
  - PSUM stays fp32; the 1/eff scale is applied per-core-exactly at the
    PSUM->SBUF copy on the DVE (Pool cannot access PSUM; anything queued
    on ACT blocks its ring-B load issues, measured +19us of DMA gaps).
  - Measured 217-220us unthrottled (vs 256-269us baseline); the device
    power-throttles DMA to ~310-330 GB/s when hot, adding up to ~25us
    run-to-run -- compute sits at <=145us per engine so the kernel stays
    DMA-bound either way.
"""

import os

import numpy as np

import concourse.bacc as bacc
import concourse.mybir as mybir
from concourse.tile import TileContext
from concourse.bass_utils import run_bass_kernel_spmd

B, L, D = 128, 1024, 2048
NCORES = 8
SLOTS = B // NCORES  # 16
PCHUNK = 128         # rows per chunk (partition dim of a tile)
NTILE = 512          # matmul moving free dim (one PSUM bank of fp32)

TILE_BUFS = int(os.environ.get("TILE_BUFS", "10"))
KCAP = int(os.environ.get("KCAP", "48"))  # max chunks on the DVE add path
REDUCE_EARLY = os.environ.get("REDUCE_EARLY", "0") == "1"

LAST_RESULTS = None  # BassKernelResults of the most recent device run


def _plan(eff):
    """Snake-assign sorted batches to cores; order slots by ascending max."""
    order = np.argsort(-eff, kind="stable")
    cores = [[] for _ in range(NCORES)]
    for i, idx in enumerate(order):
        blk, pos = divmod(i, NCORES)
        c = pos if blk % 2 == 0 else NCORES - 1 - pos
        cores[c].append(int(idx))
    smax = [max(int(eff[cores[c][s]]) for c in range(NCORES)) for s in range(SLOTS)]
    smin = [min(int(eff[cores[c][s]]) for c in range(NCORES)) for s in range(SLOTS)]
    sorder = sorted(range(SLOTS), key=lambda s: smax[s])
    cores = [[cores[c][s] for s in sorder] for c in range(NCORES)]
    smax = tuple(smax[s] for s in sorder)
    smin = tuple(smin[s] for s in sorder)
    return cores, smax, smin


def _fragments(smax, smin):
    """Chunk the packed [T, D] stream into (chunk, slot) fragments.

    Returns (T, offsets g, frags, n_pe).  frags[i] = (c, s, a, b, r0, fi):
    partitions [a,b) of chunk c hold slot-local rows [r0, r0 + b - a) of
    slot s; fi is the mask-column index for PE fragments or None for
    uniform chunks that take the DVE-accumulator path."""
    g = [0]
    for m in smax:
        g.append(g[-1] + m)
    T = g[-1]
    nchunk = -(-T // PCHUNK)
    raw, s = [], 0
    for c in range(nchunk):
        lo, hi = c * PCHUNK, min((c + 1) * PCHUNK, T)
        while g[s + 1] <= lo:
            s += 1
        si = s
        while si < SLOTS and g[si] < hi:
            a = max(lo, g[si]) - lo
            b = min(hi, g[si + 1]) - lo
            raw.append((c, si, a, b, max(lo, g[si]) - g[si]))
            si += 1
    # Uniform chunks: whole 128-row chunk inside one slot, and every core
    # has all its rows valid (r0 + 128 <= smin) -- no mask needed.  The
    # final slot stays entirely on the PE so the drain after the last DMA
    # is one matmul group + scale + out, not an add+reduce chain.
    uni = [
        i
        for i, (c, s, a, b, r0) in enumerate(raw)
        if b - a == PCHUNK and r0 + PCHUNK <= smin[s] and s < SLOTS - 1
    ]
    if len(uni) > KCAP:  # keep the DVE under the DMA floor; demote evenly
        keep = {uni[(i * len(uni)) // KCAP] for i in range(KCAP)}
        uni = [i for i in uni if i in keep]
    uni = set(uni)
    frags, n_pe = [], 0
    for i, (c, s, a, b, r0) in enumerate(raw):
        if i in uni:
            frags.append((c, s, a, b, r0, None))
        else:
            frags.append((c, s, a, b, r0, n_pe))
            n_pe += 1
    return T, g, frags, n_pe


_PROGRAM_CACHE = {}


def _build_program(smax, smin):
    T, _, frags, n_pe = _fragments(smax, smin)
    ones_col = n_pe  # shared all-ones column for accumulator reduces
    by_chunk = {}
    for c, s, a, b, r0, fi in frags:
        by_chunk.setdefault(c, []).append((s, a, b, r0, fi))
    slot_last_chunk = {}
    slot_last_uni = {}  # slot -> chunk of its last uniform (DVE-path) chunk
    uni_count = {}
    for c, s, a, b, r0, fi in frags:
        slot_last_chunk[s] = max(slot_last_chunk.get(s, -1), c)
        if fi is None:
            slot_last_uni[s] = max(slot_last_uni.get(s, -1), c)
            uni_count[s] = uni_count.get(s, 0) + 1
    # A single uniform chunk goes through the PE with the shared ones
    # column -- an accumulator would cost a DVE copy plus a PE reduce.
    has_acc = {s for s, n in uni_count.items() if n >= 2}
    reduce_at = {
        s: (slot_last_uni[s] if REDUCE_EARLY else slot_last_chunk[s])
        for s in has_acc
    }
    # The slot's final PE matmul (gets stop=True): the reduce if no PE
    # fragment follows it, else the last fragment / uniform-PE group.
    slot_stop_on_reduce = {
        s: reduce_at[s] == slot_last_chunk[s] for s in has_acc
    }

    # Bacc (not raw Bass): its compile pass splits multi-sem waits and moves
    # matmul waits onto ldweights -- walrus allows only 1 wait per instruction.
    nc = bacc.Bacc(None, target_bir_lowering=False)
    f32 = mybir.dt.float32
    f32r = mybir.dt.float32r
    packed = nc.dram_tensor("packed", [T, D], f32r, kind="ExternalInput")
    maskt = nc.dram_tensor("maskt", [PCHUNK, n_pe + 1], f32r, kind="ExternalInput")
    scalef = nc.dram_tensor("scalef", [1, SLOTS], f32, kind="ExternalInput")
    out = nc.dram_tensor("out", [SLOTS, D], f32, kind="ExternalOutput")

    with TileContext(nc) as tc:
        with (
            tc.tile_pool(name="mask", bufs=1) as mpool,
            tc.tile_pool(name="scale", bufs=1) as spool,
            tc.tile_pool(name="tiles", bufs=TILE_BUFS) as tpool,
            tc.tile_pool(name="accs", bufs=2) as apool,
            tc.tile_pool(name="psum", bufs=4, space="PSUM") as ppool,
            tc.tile_pool(name="outs", bufs=3) as opool,
        ):
            mask_tile = mpool.tile([PCHUNK, n_pe + 1], f32r)
            scale_tile = spool.tile([1, SLOTS], f32)
            dma_engines = [nc.sync, nc.scalar]
            psums = {}    # slot -> psum tile [2, D//2]
            started = set()
            accs = {}     # slot -> acc tile
            pend = {}     # slot -> (tile, off) first uniform chunk, unfused

            def mm_group(s, lhsT, rhs_tile, off, rows, stop):
                if s not in psums:
                    psums[s] = (
                        ppool.tile([1, D // 2], f32, name="psum_a", tag="ps"),
                        ppool.tile([1, D // 2], f32, name="psum_b", tag="ps"),
                    )
                pa, pb = psums[s]
                half = [pa, pa, pb, pb]
                start = s not in started
                started.add(s)
                for j in range(D // NTILE):
                    nc.tensor.matmul(
                        half[j][0:1, (j % 2) * NTILE : (j % 2 + 1) * NTILE],
                        lhsT,
                        rhs_tile[0:rows, off + j * NTILE : off + (j + 1) * NTILE],
                        start=start,
                        stop=stop,
                    )

            def consume_chunk(c, tile, off, rows):
                done = []
                for s, a, b, r0, fi in by_chunk[c]:
                    if fi is None and s not in has_acc:
                        # Lone uniform chunk: ones-column PE group beats a
                        # DVE copy + PE reduce.
                        mm_group(
                            s,
                            mask_tile[0:PCHUNK, ones_col : ones_col + 1],
                            tile,
                            off,
                            PCHUNK,
                            stop=(c == slot_last_chunk[s]),
                        )
                    elif fi is None:
                        if s in pend:
                            # Fused init: acc = chunk1 + chunk2 in one op
                            # instead of copy-then-add.
                            t0, o0 = pend.pop(s)
                            accs[s] = apool.tile(
                                [PCHUNK, D], f32r, name="acc", tag="a"
                            )
                            nc.vector.tensor_add(
                                out=accs[s][:],
                                in0=t0[:, o0 : o0 + D],
                                in1=tile[:, off : off + D],
                            )
                        elif s not in accs:
                            pend[s] = (tile, off)
                        else:
                            nc.vector.tensor_add(
                                out=accs[s][:],
                                in0=accs[s][:],
                                in1=tile[:, off : off + D],
                            )
                    else:
                        mm_group(
                            s,
                            mask_tile[0:rows, fi : fi + 1],
                            tile,
                            off,
                            rows,
                            stop=(
                                c == slot_last_chunk[s]
                                and not slot_stop_on_reduce.get(s, False)
                            ),
                        )
                    if c == reduce_at.get(s, -1):
                        # Reduce once the accumulator is complete (at slot
                        # end by default: issuing it earlier serializes the
                        # PE queue behind the DVE add chain, measured +12us).
                        mm_group(
                            s,
                            mask_tile[0:PCHUNK, ones_col : ones_col + 1],
                            accs.pop(s),
                            0,
                            PCHUNK,
                            stop=slot_stop_on_reduce[s],
                        )
                    if c == slot_last_chunk[s]:
                        done.append(s)
                for s in done:
                    pa, pb = psums.pop(s)
                    # DVE for PSUM->SBUF (Pool can't access PSUM, and any
                    # op queued on ACT blocks its ring-B load issues -- that
                    # cost 19us of mid-run DMA gaps when measured).
                    sc = scale_tile[0:1, s : s + 1]
                    ot = opool.tile([1, D], f32)
                    nc.vector.tensor_scalar_mul(
                        out=ot[:, 0 : D // 2], in0=pa[:], scalar1=sc
                    )
                    nc.vector.tensor_scalar_mul(
                        out=ot[:, D // 2 : D], in0=pb[:], scalar1=sc
                    )
                    nc.sync.dma_start(out=out[s : s + 1, :], in_=ot[:])

            def load_pair(c):
                pair = tpool.tile([PCHUNK, 2 * D], f32r, name="pair", tag="t")
                src = packed[c * PCHUNK : (c + 2) * PCHUNK, :].rearrange(
                    "(c p) d -> p c d", p=PCHUNK
                )
                dst = pair[:].rearrange("p (c d) -> p c d", c=2)
                dma_engines[c // 2 % 2].dma_start(out=dst, in_=src)
                return pair

            c = 0
            nchunk = -(-T // PCHUNK)
            if T >= 4 * PCHUNK:
                # Issue the first two pair loads before the mask DMAs: the
                # big loads start ~3us earlier, and the first matmuls (tiny
                # early slots) have slack to wait for the masks behind them.
                p0, p1 = load_pair(0), load_pair(2)
                nc.sync.dma_start(out=mask_tile[:], in_=maskt[:])
                nc.scalar.dma_start(out=scale_tile[:], in_=scalef[:])
                for cc, (pp, off) in enumerate(
                    [(p0, 0), (p0, D), (p1, 0), (p1, D)]
                ):
                    consume_chunk(cc, pp, off, PCHUNK)
                c = 4
            else:
                nc.sync.dma_start(out=mask_tile[:], in_=maskt[:])
                nc.scalar.dma_start(out=scale_tile[:], in_=scalef[:])
            while c < nchunk:
                if T - c * PCHUNK >= 2 * PCHUNK:
                    pair = load_pair(c)
                    consume_chunk(c, pair, 0, PCHUNK)
                    consume_chunk(c + 1, pair, D, PCHUNK)
                    c += 2
                else:
                    rows = T - c * PCHUNK
                    single = tpool.tile([PCHUNK, D], f32r, name="single", tag="t")
                    dma_engines[c // 2 % 2].dma_start(
                        out=single[0:rows, :],
                        in_=packed[c * PCHUNK : c * PCHUNK + rows, :],
                    )
                    consume_chunk(c, single, 0, rows)
                    c += 1
    nc.finalize()
    return nc


def kernel(features, lengths):
    global LAST_RESULTS
    features = np.ascontiguousarray(features, dtype=np.float32)
    lengths = np.ascontiguousarray(lengths, dtype=np.int32)
    eff = np.where(lengths > 0, lengths, L).astype(np.int64)

    cores, smax, smin = _plan(eff)
    key = (smax, smin, TILE_BUFS, KCAP)
    if key not in _PROGRAM_CACHE:
        _PROGRAM_CACHE[key] = _build_program(smax, smin)
    nc = _PROGRAM_CACHE[key]
    T, g, frags, n_pe = _fragments(smax, smin)

    in_maps = []
    for c in range(NCORES):
        perm = cores[c]
        packed = np.empty((T, D), dtype=np.float32)
        for s, bi in enumerate(perm):
            packed[g[s] : g[s + 1]] = features[bi, : smax[s], :]
        maskt = np.zeros((PCHUNK, n_pe + 1), dtype=np.float32)
        for _, s, a, b, r0, fi in frags:
            if fi is None:
                continue
            v = min(max(int(eff[perm[s]]) - r0, 0), b - a)
            if v > 0:
                maskt[a : a + v, fi] = 1.0
        maskt[:, n_pe] = 1.0
        scalef = np.zeros((1, SLOTS), dtype=np.float32)
        for s in range(SLOTS):
            scalef[0, s] = np.float32(1.0 / eff[perm[s]])
        in_maps.append({"packed": packed, "maskt": maskt, "scalef": scalef})

    trace = os.environ.get("KERNEL_TRACE", "0") == "1"
    LAST_RESULTS = run_bass_kernel_spmd(
        nc,
        in_maps,
        core_ids=list(range(NCORES)),
        trace=trace,
        trace_cores=[0] if trace else None,
    )

    out = np.empty((B, D), dtype=np.float32)
    for c in range(NCORES):
        out[np.asarray(cores[c])] = LAST_RESULTS.results[c]["out"]
    return out


# revision 46
# speedup vs baseline: 1.1106x; 1.0297x over previous
"""Variable-length average pooling (prefix mean over seq axis) on 8 trn2 cores.

Strategy (pure data parallelism over batch, dense packed layout):
  - eff_len[b] = lengths[b] if >0 else L.  pooled[b] = sum_{l<eff} x[b,l,:] / eff.
  - Sort batches by eff_len desc, snake-assign 16 per core; slot s's baked
    row budget is smax[s] = max over cores of that slot's eff (sorted
    grouping keeps the spread small, ~5% over the per-core ideal).
  - Host packs each core's rows 0..smax[s] of every slot back-to-back into
    one dense [T, D] buffer (T = sum smax ~ 9019 rows, 73.9 MB vs the 82 MB
    a 128-aligned per-slot layout loads).  Slots are ordered by ascending
    smax: the tiny-slot psum churn lands in the DMA ramp-up phase and the
    kernel drains on one long all-PE slot (one matmul group + copy + out).
  - The SPMD program streams T rows as 128-row chunks (2 MB pair DMAs
    alternating the two HWDGE rings SP/ACT; 16 SDMA engines saturate at
    ~360-370 GB/s unthrottled, so the DMA floor is ~203 us and every
    compute engine must stay under it).  The first two pairs are issued
    before the mask DMAs; 10 pair buffers (20 MB SBUF) ride out transient
    consumer backlogs that otherwise stall the rings on buffer reuse.
  - Tensors are declared float32r: at moving free dim 512 the PE runs
    1 cycle/row vs 4 for exact fp32 (bits are identical to fp32 on the
    host; extra rounding error measured at 1e-4 vs the 2e-2 gate).  A DMA
    into an f32r dram tensor satisfies the BIR verifier's "rounded to
    FP32r" rule; bitcasting an f32 tile at the matmul does not.
  - Work split so no engine approaches the DMA floor (measured: PE 2.36us
    per 4-matmul fragment group, DVE add 2.28us, all-PE was 206us busy):
      * chunks fully inside one slot and below the slot's min core eff
        ("uniform": every row valid on every core) accumulate into a
        per-slot SBUF tile on the DVE; one PE pass per slot (ones-column
        matmul, issued at slot end -- issuing it earlier serializes the
        PE queue behind the DVE add chain, measured +12us) reduces it.
      * boundary / ragged fragments go straight to the PE as
        psum[1,512] += col[128,1].T @ chunk[128,512] with host-built 0/1
        columns (zero outside the fragment or past the core's own eff),
        accumulated across a slot's chunks in two 2-bank psum tiles
        (a single 4-bank [1,2048] psum tile breaks accumulation).
  - PSUM stays fp32; the 1/eff scale is applied per-core-exactly at the
    PSUM->SBUF copy on the DVE (Pool cannot access PSUM; anything queued
    on ACT blocks its ring-B load issues, measured +19us of DMA gaps).
  - Measured 217-220us unthrottled (vs 256-269us baseline); the device
    power-throttles DMA to ~310-330 GB/s when hot, adding up to ~25us
    run-to-run -- compute sits at <=145us per engine so the kernel stays
    DMA-bound either way.
"""

import os

import numpy as np

import concourse.bacc as bacc
import concourse.mybir as mybir
from concourse.tile import TileContext
from concourse.bass_utils import run_bass_kernel_spmd

B, L, D = 128, 1024, 2048
NCORES = 8
SLOTS = B // NCORES  # 16
PCHUNK = 128         # rows per chunk (partition dim of a tile)
NTILE = 512          # matmul moving free dim (one PSUM bank of fp32)

TILE_BUFS = int(os.environ.get("TILE_BUFS", "10"))
KCAP = int(os.environ.get("KCAP", "48"))  # max chunks on the DVE add path
REDUCE_EARLY = os.environ.get("REDUCE_EARLY", "0") == "1"

LAST_RESULTS = None  # BassKernelResults of the most recent device run


def _plan(eff):
    """Snake-assign sorted batches to cores; order slots by ascending max."""
    order = np.argsort(-eff, kind="stable")
    cores = [[] for _ in range(NCORES)]
    for i, idx in enumerate(order):
        blk, pos = divmod(i, NCORES)
        c = pos if blk % 2 == 0 else NCORES - 1 - pos
        cores[c].append(int(idx))
    smax = [max(int(eff[cores[c][s]]) for c in range(NCORES)) for s in range(SLOTS)]
    smin = [min(int(eff[cores[c][s]]) for c in range(NCORES)) for s in range(SLOTS)]
    sorder = sorted(range(SLOTS), key=lambda s: smax[s])
    cores = [[cores[c][s] for s in sorder] for c in range(NCORES)]
    smax = tuple(smax[s] for s in sorder)
    smin = tuple(smin[s] for s in sorder)
    return cores, smax, smin


def _fragments(smax, smin):
    """Chunk the packed [T, D] stream into (chunk, slot) fragments.

    Returns (T, offsets g, frags, n_pe).  frags[i] = (c, s, a, b, r0, fi):
    partitions [a,b) of chunk c hold slot-local rows [r0, r0 + b - a) of
    slot s; fi is the mask-column index for PE fragments or None for
    uniform chunks that take the DVE-accumulator path."""
    g = [0]
    for m in smax:
        g.append(g[-1] + m)
    T = g[-1]
    nchunk = -(-T // PCHUNK)
    raw, s = [], 0
    for c in range(nchunk):
        lo, hi = c * PCHUNK, min((c + 1) * PCHUNK, T)
        while g[s + 1] <= lo:
            s += 1
        si = s
        while si < SLOTS and g[si] < hi:
            a = max(lo, g[si]) - lo
            b = min(hi, g[si + 1]) - lo
            raw.append((c, si, a, b, max(lo, g[si]) - g[si]))
            si += 1
    # Uniform chunks: whole 128-row chunk inside one slot, and every core
    # has all its rows valid (r0 + 128 <= smin) -- no mask needed.  The
    # final slot stays entirely on the PE so the drain after the last DMA
    # is one matmul group + scale + out, not an add+reduce chain.
    uni = [
        i
        for i, (c, s, a, b, r0) in enumerate(raw)
        if b - a == PCHUNK and r0 + PCHUNK <= smin[s] and s < SLOTS - 1
    ]
    if len(uni) > KCAP:  # keep the DVE under the DMA floor; demote evenly
        keep = {uni[(i * len(uni)) // KCAP] for i in range(KCAP)}
        uni = [i for i in uni if i in keep]
    uni = set(uni)
    frags, n_pe = [], 0
    for i, (c, s, a, b, r0) in enumerate(raw):
        if i in uni:
            frags.append((c, s, a, b, r0, None))
        else:
            frags.append((c, s, a, b, r0, n_pe))
            n_pe += 1
    return T, g, frags, n_pe


_PROGRAM_CACHE = {}


def _build_program(smax, smin):
    T, _, frags, n_pe = _fragments(smax, smin)
    ones_col = n_pe  # shared all-ones column for accumulator reduces
    by_chunk = {}
    for c, s, a, b, r0, fi in frags:
        by_chunk.setdefault(c, []).append((s, a, b, r0, fi))
    slot_last_chunk = {}
    slot_last_uni = {}  # slot -> chunk of its last uniform (DVE-path) chunk
    uni_count = {}
    for c, s, a, b, r0, fi in frags:
        slot_last_chunk[s] = max(slot_last_chunk.get(s, -1), c)
        if fi is None:
            slot_last_uni[s] = max(slot_last_uni.get(s, -1), c)
            uni_count[s] = uni_count.get(s, 0) + 1
    # A single uniform chunk goes through the PE with the shared ones
    # column -- an accumulator would cost a DVE copy plus a PE reduce.
    has_acc = {s for s, n in uni_count.items() if n >= 2}
    reduce_at = {
        s: (slot_last_uni[s] if REDUCE_EARLY else slot_last_chunk[s])
        for s in has_acc
    }
    # The slot's final PE matmul (gets stop=True): the reduce if no PE
    # fragment follows it, else the last fragment / uniform-PE group.
    slot_stop_on_reduce = {
        s: reduce_at[s] == slot_last_chunk[s] for s in has_acc
    }

    # Bacc (not raw Bass): its compile pass splits multi-sem waits and moves
    # matmul waits onto ldweights -- walrus allows only 1 wait per instruction.
    nc = bacc.Bacc(None, target_bir_lowering=False)
    f32 = mybir.dt.float32
    f32r = mybir.dt.float32r
    packed = nc.dram_tensor("packed", [T, D], f32r, kind="ExternalInput")
    maskt = nc.dram_tensor("maskt", [PCHUNK, n_pe + 1], f32r, kind="ExternalInput")
    scalef = nc.dram_tensor("scalef", [1, SLOTS], f32, kind="ExternalInput")
    out = nc.dram_tensor("out", [SLOTS, D], f32, kind="ExternalOutput")

    with TileContext(nc) as tc:
        with (
            tc.tile_pool(name="mask", bufs=1) as mpool,
            tc.tile_pool(name="scale", bufs=1) as spool,
            tc.tile_pool(name="tiles", bufs=TILE_BUFS) as tpool,
            tc.tile_pool(name="accs", bufs=2) as apool,
            tc.tile_pool(name="psum", bufs=4, space="PSUM") as ppool,
            tc.tile_pool(name="outs", bufs=3) as opool,
        ):
            mask_tile = mpool.tile([PCHUNK, n_pe + 1], f32r)
            scale_tile = spool.tile([1, SLOTS], f32)
            dma_engines = [nc.sync, nc.scalar]
            psums = {}    # slot -> psum tile [2, D//2]
            started = set()
            accs = {}     # slot -> acc tile
            pend = {}     # slot -> (tile, off) first uniform chunk, unfused

            def mm_group(s, lhsT, rhs_tile, off, rows, stop):
                if s not in psums:
                    psums[s] = (
                        ppool.tile([1, D // 2], f32, name="psum_a", tag="ps"),
                        ppool.tile([1, D // 2], f32, name="psum_b", tag="ps"),
                    )
                pa, pb = psums[s]
                half = [pa, pa, pb, pb]
                start = s not in started
                started.add(s)
                for j in range(D // NTILE):
                    nc.tensor.matmul(
                        half[j][0:1, (j % 2) * NTILE : (j % 2 + 1) * NTILE],
                        lhsT,
                        rhs_tile[0:rows, off + j * NTILE : off + (j + 1) * NTILE],
                        start=start,
                        stop=stop,
                    )

            def consume_chunk(c, tile, off, rows):
                done = []
                for s, a, b, r0, fi in by_chunk[c]:
                    if fi is None and s not in has_acc:
                        # Lone uniform chunk: ones-column PE group beats a
                        # DVE copy + PE reduce.
                        mm_group(
                            s,
                            mask_tile[0:PCHUNK, ones_col : ones_col + 1],
                            tile,
                            off,
                            PCHUNK,
                            stop=(c == slot_last_chunk[s]),
                        )
                    elif fi is None:
                        if s in pend:
                            # Fused init: acc = chunk1 + chunk2 in one op
                            # instead of copy-then-add.
                            t0, o0 = pend.pop(s)
                            accs[s] = apool.tile(
                                [PCHUNK, D], f32r, name="acc", tag="a"
                            )
                            nc.vector.tensor_add(
                                out=accs[s][:],
                                in0=t0[:, o0 : o0 + D],
                                in1=tile[:, off : off + D],
                            )
                        elif s not in accs:
                            pend[s] = (tile, off)
                        else:
                            nc.vector.tensor_add(
                                out=accs[s][:],
                                in0=accs[s][:],
                                in1=tile[:, off : off + D],
                            )
                    else:
                        mm_group(
                            s,
                            mask_tile[0:rows, fi : fi + 1],
                            tile,
                            off,
                            rows,
                            stop=(
                                c == slot_last_chunk[s]
                                and not slot_stop_on_reduce.get(s, False)
                            ),
                        )
                    if c == reduce_at.get(s, -1):
                        # Reduce once the accumulator is complete (at slot
                        # end by default: issuing it earlier serializes the
                        # PE queue behind the DVE add chain, measured +12us).
                        mm_group(
                            s,
                            mask_tile[0:PCHUNK, ones_col : ones_col + 1],
                            accs.pop(s),
                            0,
                            PCHUNK,
                            stop=slot_stop_on_reduce[s],
                        )
                    if c == slot_last_chunk[s]:
                        done.append(s)
                for s in done:
                    pa, pb = psums.pop(s)
                    # DVE for PSUM->SBUF (Pool can't access PSUM, and any
                    # op queued on ACT blocks its ring-B load issues -- that
                    # cost 19us of mid-run DMA gaps when measured).
                    sc = scale_tile[0:1, s : s + 1]
                    ot = opool.tile([1, D], f32)
                    nc.vector.tensor_scalar_mul(
                        out=ot[:, 0 : D // 2], in0=pa[:], scalar1=sc
                    )
                    nc.vector.tensor_scalar_mul(
                        out=ot[:, D // 2 : D], in0=pb[:], scalar1=sc
                    )
                    # Outs ride the Pool SWDGE queue: an out DMA on a load
                    # ring waits for its DVE copy and stalls every load
                    # queued behind it (the in-order ring was the source of
                    # the ~2us mid-run DMA gaps at each slot close).
                    nc.gpsimd.dma_start(out=out[s : s + 1, :], in_=ot[:])

            def load_pair(c):
                pair = tpool.tile([PCHUNK, 2 * D], f32r, name="pair", tag="t")
                src = packed[c * PCHUNK : (c + 2) * PCHUNK, :].rearrange(
                    "(c p) d -> p c d", p=PCHUNK
                )
                dst = pair[:].rearrange("p (c d) -> p c d", c=2)
                dma_engines[c // 2 % 2].dma_start(out=dst, in_=src)
                return pair

            c = 0
            nchunk = -(-T // PCHUNK)
            if T >= 4 * PCHUNK:
                # Issue the first two pair loads before the mask DMAs: the
                # big loads start ~3us earlier, and the first matmuls (tiny
                # early slots) have slack to wait for the masks behind them.
                p0, p1 = load_pair(0), load_pair(2)
                nc.sync.dma_start(out=mask_tile[:], in_=maskt[:])
                nc.scalar.dma_start(out=scale_tile[:], in_=scalef[:])
                for cc, (pp, off) in enumerate(
                    [(p0, 0), (p0, D), (p1, 0), (p1, D)]
                ):
                    consume_chunk(cc, pp, off, PCHUNK)
                c = 4
            else:
                nc.sync.dma_start(out=mask_tile[:], in_=maskt[:])
                nc.scalar.dma_start(out=scale_tile[:], in_=scalef[:])
            while c < nchunk:
                if T - c * PCHUNK >= 2 * PCHUNK:
                    pair = load_pair(c)
                    consume_chunk(c, pair, 0, PCHUNK)
                    consume_chunk(c + 1, pair, D, PCHUNK)
                    c += 2
                else:
                    rows = T - c * PCHUNK
                    single = tpool.tile([PCHUNK, D], f32r, name="single", tag="t")
                    dma_engines[c // 2 % 2].dma_start(
                        out=single[0:rows, :],
                        in_=packed[c * PCHUNK : c * PCHUNK + rows, :],
                    )
                    consume_chunk(c, single, 0, rows)
                    c += 1
    nc.finalize()
    return nc


def kernel(features, lengths):
    global LAST_RESULTS
    features = np.ascontiguousarray(features, dtype=np.float32)
    lengths = np.ascontiguousarray(lengths, dtype=np.int32)
    eff = np.where(lengths > 0, lengths, L).astype(np.int64)

    cores, smax, smin = _plan(eff)
    key = (smax, smin, TILE_BUFS, KCAP)
    if key not in _PROGRAM_CACHE:
        _PROGRAM_CACHE[key] = _build_program(smax, smin)
    nc = _PROGRAM_CACHE[key]
    T, g, frags, n_pe = _fragments(smax, smin)

    in_maps = []
    for c in range(NCORES):
        perm = cores[c]
        packed = np.empty((T, D), dtype=np.float32)
        for s, bi in enumerate(perm):
            packed[g[s] : g[s + 1]] = features[bi, : smax[s], :]
        maskt = np.zeros((PCHUNK, n_pe + 1), dtype=np.float32)
        for _, s, a, b, r0, fi in frags:
            if fi is None:
                continue
            v = min(max(int(eff[perm[s]]) - r0, 0), b - a)
            if v > 0:
                maskt[a : a + v, fi] = 1.0
        maskt[:, n_pe] = 1.0
        scalef = np.zeros((1, SLOTS), dtype=np.float32)
        for s in range(SLOTS):
            scalef[0, s] = np.float32(1.0 / eff[perm[s]])
        in_maps.append({"packed": packed, "maskt": maskt, "scalef": scalef})

    trace = os.environ.get("KERNEL_TRACE", "0") == "1"
    LAST_RESULTS = run_bass_kernel_spmd(
        nc,
        in_maps,
        core_ids=list(range(NCORES)),
        trace=trace,
        trace_cores=[0] if trace else None,
    )

    out = np.empty((B, D), dtype=np.float32)
    for c in range(NCORES):
        out[np.asarray(cores[c])] = LAST_RESULTS.results[c]["out"]
    return out
